# revision 3
# baseline (speedup 1.0000x reference)
"""HalfKP input layer (dual GEMV + bias + relu) on 8 Trainium2 NeuronCores.

out[512] = concat(relu(W_my @ x[:41024] + b_my), relu(W_opp @ x[41024:] + b_opp))

Sharding: 512 output rows split 64 rows/core (output-feature parallel; cores
0-3 handle W_my, 4-7 handle W_opp).  Per core the [64, 41024] shard is
host-repacked into [128, 16*1282]: partition p = rr*32 + b holds row
(t*4 + rr)'s k-block b (kb=1282) at free offset t*1282.  The device streams W
contiguously at full DMA bandwidth, runs 16 fused multiply+reduce DVE ops
(tensor_tensor_reduce against a [128, 1282] x-block tile, bias in the seed),
contracts the 32 k-block partials per row with one tiny PE matmul, applies
relu on ACT, and writes a [4, 16] result per core.
"""

import numpy as np

K = 41024          # features per side
B = 32             # k-blocks per row
KB = K // B        # 1282 elements per k-block
R = 128 // B       # 4 rows processed per DVE op
T = 64 // R        # 16 DVE ops (row groups) per core
N_CORES = 8
ROWS_PER_CORE = 64

_compiled = None


def _build_nc():
    import concourse.bacc as bacc
    import concourse.mybir as mybir
    import concourse.tile as tile
    from concourse.dve_ops import TENSOR_TENSOR_REDUCE

    F32 = mybir.dt.float32

    nc = bacc.Bacc("TRN2", target_bir_lowering=False, debug=False)

    wt_d = nc.dram_tensor("wt", [128, T * KB], F32, kind="ExternalInput")
    xq_d = nc.dram_tensor("xq", [128, KB], F32, kind="ExternalInput")
    mask_d = nc.dram_tensor("mask", [128, R], F32, kind="ExternalInput")
    seed_d = nc.dram_tensor("seed", [128, T], F32, kind="ExternalInput")
    out_d = nc.dram_tensor("out", [R, T], F32, kind="ExternalOutput")

    with tile.TileContext(nc) as tc:
        with (
            tc.tile_pool(name="const", bufs=1) as constp,
            tc.tile_pool(name="w", bufs=3) as wp,
            tc.tile_pool(name="scratch", bufs=1) as sp,
            tc.tile_pool(name="ps", bufs=1, space="PSUM") as psp,
        ):
            xq_sb = constp.tile([128, KB], F32, tag="xq")
            nc.sync.dma_start(xq_sb[:], xq_d[:])
            mask_sb = constp.tile([128, R], F32, tag="mask")
            nc.sync.dma_start(mask_sb[:], mask_d[:])
            seed_sb = constp.tile([128, T], F32, tag="seed")
            nc.sync.dma_start(seed_sb[:], seed_d[:])

            acc = constp.tile([128, T], F32, tag="acc")
            prod = sp.tile([128, KB], F32, tag="prod")
            for t in range(T):
                w_sb = wp.tile([128, KB], F32, tag="w")
                nc.sync.dma_start(w_sb[:], wt_d[:, t * KB : (t + 1) * KB])
                # custom-DVE fused multiply+reduce (ships its ucode table in
                # the NEFF; the native InstTensorTensorReduce opcode faults at
                # runtime without it): out = in0*in1*s1; accum = s0 + sum(out)
                nc.vector._custom_dve(
                    TENSOR_TENSOR_REDUCE,
                    out=prod[:],
                    in0=w_sb[:],
                    in1=xq_sb[:],
                    s0=seed_sb[:, t : t + 1],
                    s1=1.0,
                    accum_out=acc[:, t : t + 1],
                )

            ps = psp.tile([R, T], F32, tag="ps")
            nc.tensor.matmul(ps[:], lhsT=mask_sb[:], rhs=acc[:], start=True, stop=True)
            out_sb = sp.tile([R, T], F32, tag="out")
            nc.scalar.activation(
                out_sb[:], ps[:], mybir.ActivationFunctionType.Relu
            )
            nc.sync.dma_start(out_d[:], out_sb[:])

    nc.compile()
    return nc


def _get_nc():
    global _compiled
    if _compiled is None:
        _compiled = _build_nc()
    return _compiled


def make_in_maps(input, W_my, b_my, W_opp, b_opp):
    """Host-side sharding: per-core input dicts."""
    x = np.ascontiguousarray(input, dtype=np.float32)
    Wcat = np.concatenate(
        [np.asarray(W_my, np.float32), np.asarray(W_opp, np.float32)], axis=0
    )
    bcat = np.concatenate(
        [np.asarray(b_my, np.float32), np.asarray(b_opp, np.float32)]
    )

    mask = (np.arange(128)[:, None] // B == np.arange(R)[None, :]).astype(np.float32)

    in_maps = []
    for c in range(N_CORES):
        Wsh = Wcat[c * ROWS_PER_CORE : (c + 1) * ROWS_PER_CORE]  # [64, K]
        xs = x[:K] if c < 4 else x[K:]
        # wt[p = rr*B + b, t*KB + j] = Wsh[t*R + rr, b*KB + j]
        wt = np.ascontiguousarray(
            Wsh.reshape(T, R, B, KB).transpose(1, 2, 0, 3).reshape(128, T * KB)
        )
        xq = np.ascontiguousarray(np.tile(xs.reshape(B, KB), (R, 1)))  # [128, KB]
        bsh = bcat[c * ROWS_PER_CORE : (c + 1) * ROWS_PER_CORE]
        seed = np.zeros((128, T), np.float32)
        # partition rr*B (b == 0) seeds the bias for row t*R + rr
        seed[np.arange(R) * B, :] = bsh.reshape(T, R).T
        in_maps.append({"wt": wt, "xq": xq, "mask": mask, "seed": seed})
    return in_maps


def gather_output(results):
    """results: list of per-core dicts with 'out' [R, T] -> full [512]."""
    outs = []
    for c in range(N_CORES):
        o = np.asarray(results[c]["out"], np.float32)  # [R, T]
        outs.append(o.T.ravel())  # row r = t*R + rr
    return np.concatenate(outs)


def run_on_hw(in_maps, trace=False, **kwargs):
    from concourse.bass_utils import run_bass_kernel_spmd

    nc = _get_nc()
    return run_bass_kernel_spmd(
        nc, in_maps, core_ids=list(range(N_CORES)), trace=trace, **kwargs
    )


def kernel(input, W_my, b_my, W_opp, b_opp):
    in_maps = make_in_maps(input, W_my, b_my, W_opp, b_opp)
    res = run_on_hw(in_maps)
    return gather_output(res.results)


# revision 4
# speedup vs baseline: 1.2592x; 1.2592x over previous
"""HalfKP input layer (dual GEMV + bias + relu) on 8 Trainium2 NeuronCores.

out[512] = concat(relu(W_my @ x[:41024] + b_my), relu(W_opp @ x[41024:] + b_opp))

Sharding: 512 output rows split 64 rows/core (output-feature parallel; cores
0-3 handle W_my, 4-7 handle W_opp).  Per core the [64, 41024] shard is
host-repacked into [128, 8*2564]: partition p = rr*16 + b holds row
(t*8 + rr)'s k-block b (kb=2564) at free offset t*2564.  The device streams W
contiguously (10.3KB runs per partition per DMA), runs 8 fused
multiply+reduce custom-DVE ops (TENSOR_TENSOR_REDUCE against a [128, 2564]
x-block tile, bias seeded via s0), contracts the 16 k-block partials per row
with one tiny PE matmul, applies relu on DVE, and writes a [8, 8] result per
core.  Memory-roofline bound: ~10.5 MB HBM reads per core.
"""

import numpy as np

K = 41024          # features per side
B = 16             # k-blocks per row
KB = K // B        # 2564 elements per k-block
HKB = KB // 2      # 1282: last chunk is split in half to shorten the tail
R = 128 // B       # 8 rows processed per DVE op
T = 64 // R        # 8 DVE ops (row groups) per core
XCOLS = KB + 2 * T  # xq | mask[8] | seed[8]
N_CORES = 8
ROWS_PER_CORE = 64

_compiled = None


def _build_nc():
    import concourse.bacc as bacc
    import concourse.mybir as mybir
    import concourse.tile as tile
    from concourse.dve_ops import TENSOR_TENSOR_REDUCE

    F32 = mybir.dt.float32

    nc = bacc.Bacc("TRN2", target_bir_lowering=False, debug=False)

    wt_d = nc.dram_tensor("wt", [128, T * KB], F32, kind="ExternalInput")
    xqp_d = nc.dram_tensor("xqp", [128, XCOLS], F32, kind="ExternalInput")
    out_d = nc.dram_tensor("out", [R, T], F32, kind="ExternalOutput")

    def ttr(w_ap, xq_ap, seed_ap, acc_ap, prod_ap):
        # out = in0*in1*s1; accum = s0 + sum(out)  (custom-DVE ucode op)
        nc.vector._custom_dve(
            TENSOR_TENSOR_REDUCE,
            out=prod_ap,
            in0=w_ap,
            in1=xq_ap,
            s0=seed_ap,
            s1=1.0,
            accum_out=acc_ap,
        )

    with tile.TileContext(nc) as tc:
        with (
            tc.tile_pool(name="const", bufs=1) as constp,
            tc.tile_pool(name="w", bufs=T + 1) as wp,
            tc.tile_pool(name="scratch", bufs=1) as sp,
            tc.tile_pool(name="ps", bufs=1, space="PSUM") as psp,
        ):
            # constants ride the scalar (ACT) HWDGE ring, W rides sync's
            xqp = constp.tile([128, XCOLS], F32, tag="xqp")
            nc.scalar.dma_start(xqp[:], xqp_d[:])
            xq = xqp[:, 0:KB]
            mask = xqp[:, KB : KB + R]
            seed = xqp[:, KB + R : KB + R + T]

            acc = constp.tile([128, T], F32, tag="acc")
            acc_a = constp.tile([128, 1], F32, tag="acc_a")
            prod = sp.tile([128, KB], F32, tag="prod")

            for t in range(T - 1):
                w_sb = wp.tile([128, KB], F32, tag="w")
                nc.sync.dma_start(w_sb[:], wt_d[:, t * KB : (t + 1) * KB])
                ttr(w_sb[:], xq, seed[:, t : t + 1], acc[:, t : t + 1], prod[:])
            # final column split in two so the tail DVE op is half-length
            t = T - 1
            w_a = wp.tile([128, HKB], F32, tag="wh")
            nc.sync.dma_start(w_a[:], wt_d[:, t * KB : t * KB + HKB])
            w_b = wp.tile([128, HKB], F32, tag="wh")
            nc.sync.dma_start(w_b[:], wt_d[:, t * KB + HKB : (t + 1) * KB])
            ttr(w_a[:], xqp[:, 0:HKB], seed[:, t : t + 1], acc_a[:], prod[:, 0:HKB])
            ttr(
                w_b[:],
                xqp[:, HKB:KB],
                acc_a[:],
                acc[:, t : t + 1],
                prod[:, HKB:KB],
            )

            ps = psp.tile([R, T], F32, tag="ps")
            nc.tensor.matmul(ps[:], lhsT=mask, rhs=acc[:], start=True, stop=True)
            out_sb = sp.tile([R, T], F32, tag="out")
            nc.vector.tensor_scalar_max(out_sb[:], ps[:], 0.0)
            nc.sync.dma_start(out_d[:], out_sb[:])

    nc.compile()
    return nc


def _get_nc():
    global _compiled
    if _compiled is None:
        _compiled = _build_nc()
    return _compiled


def make_in_maps(input, W_my, b_my, W_opp, b_opp):
    """Host-side sharding: per-core input dicts."""
    x = np.ascontiguousarray(input, dtype=np.float32)
    Wcat = np.concatenate(
        [np.asarray(W_my, np.float32), np.asarray(W_opp, np.float32)], axis=0
    )
    bcat = np.concatenate(
        [np.asarray(b_my, np.float32), np.asarray(b_opp, np.float32)]
    )

    mask = (np.arange(128)[:, None] // B == np.arange(R)[None, :]).astype(np.float32)

    in_maps = []
    for c in range(N_CORES):
        Wsh = Wcat[c * ROWS_PER_CORE : (c + 1) * ROWS_PER_CORE]  # [64, K]
        xs = x[:K] if c < 4 else x[K:]
        # wt[p = rr*B + b, t*KB + j] = Wsh[t*R + rr, b*KB + j]
        wt = np.ascontiguousarray(
            Wsh.reshape(T, R, B, KB).transpose(1, 2, 0, 3).reshape(128, T * KB)
        )
        bsh = bcat[c * ROWS_PER_CORE : (c + 1) * ROWS_PER_CORE]
        seed = np.zeros((128, T), np.float32)
        # partition rr*B (b == 0) seeds the bias for row t*R + rr
        seed[np.arange(R) * B, :] = bsh.reshape(T, R).T
        xqp = np.empty((128, XCOLS), np.float32)
        xqp[:, 0:KB] = np.tile(xs.reshape(B, KB), (R, 1))
        xqp[:, KB : KB + R] = mask
        xqp[:, KB + R :] = seed
        in_maps.append({"wt": wt, "xqp": xqp})
    return in_maps


def gather_output(results):
    """results: list of per-core dicts with 'out' [R, T] -> full [512]."""
    outs = []
    for c in range(N_CORES):
        o = np.asarray(results[c]["out"], np.float32)  # [R, T]
        outs.append(o.T.ravel())  # row r = t*R + rr
    return np.concatenate(outs)


def run_on_hw(in_maps, trace=False, **kwargs):
    from concourse.bass_utils import run_bass_kernel_spmd

    nc = _get_nc()
    return run_bass_kernel_spmd(
        nc, in_maps, core_ids=list(range(N_CORES)), trace=trace, **kwargs
    )


def kernel(input, W_my, b_my, W_opp, b_opp):
    in_maps = make_in_maps(input, W_my, b_my, W_opp, b_opp)
    res = run_on_hw(in_maps)
    return gather_output(res.results)
